# revision 34
# baseline (speedup 1.0000x reference)
"""Trainium2 Bass kernel for nn_CustomS4 (fp8 DoubleRow redesign).

Reference pipeline:
    z   = x @ W^T + b                      adapter Linear      [B,T,D]
    xh  = LN(z) * gamma + beta             LayerNorm over D
    u   = xh @ Bm                          input projection    [B,T,N]
    h_T = sum_t u_t A^{T-1-t}              linear scan, final state only
    out = normalize_rows(h_T @ C)          [B, D]

Device formulation (per core: B_LOC=4 batches, last T_EFF=64 tokens):
  * z_x = x8 @ W8^T in fp8 e4m3 with DoubleRow perf mode (0.5 cyc/row),
    token-major [tok, 768].  ssq = row-sum(z_x^2) via ACT Square+accum /
    DVE tensor_tensor_reduce / Pool — one op per PSUM tile.
  * stats q = x~ @ [P1|m|2wb] with residual fp8: x~ = x8 + xr/16 and
    P1 = P8 + Pr/16 shipped as pre-scaled fp8 tensors so everything
    accumulates in one PSUM tile per token-half.  LN folds (c1 and
    -mu*gv outer product) are K=1 matmuls accumulated into the same tile.
  * var/s chain runs token-major on [128,2] tiles (tokens on partitions).
  * u = s * v (tensor_scalar with per-partition s), transposed back to
    feature-major by PE transpose for the chunked scan (L1=8 x L2=8).
  * norm: ||y||^2 = h^T (C C^T) h; 1/||y|| folded into h before
    y^T = C^T h' computed in [128,6,4] layout and DMA'd transposed.

Numerics validated on host: rel err ~4e-3 (tolerance 2e-2).
Sharding: data-parallel over batch, 4 per core x 8 cores, no collectives.
"""

import numpy as np

import concourse.bacc as bacc
import concourse.mybir as mybir
import concourse.tile as tile
from concourse.bass_utils import run_bass_kernel_spmd

F32 = mybir.dt.float32
BF16 = mybir.dt.bfloat16
F8 = mybir.dt.float8e4

B, T, D, N = 32, 2048, 768, 64
N_CORES = 8
B_LOC = B // N_CORES
L1 = 8
LN_EPS = 1e-5

# fp8 power-of-2 scale factors (host folds the inverses into constants)
SW = 8.0      # W
SP = 32.0     # P1
SR = 16.0     # x residual
SRP = 16.0    # P1 residual
SM = 256.0    # m (mean column)
SWB = 128.0   # 2*W^T b column

FCH = 384     # z feature chunk (PSUM tile [128, FCH] f32 fits one bank)
POOL_SLABS = 3     # filler memsets delaying the Pool-queue sb DMA

LAST_RESULTS = None
LAST_NC = None

# bisection / fallback switches
USE_DR = True        # fp8 DoubleRow matmuls (2 k-tiles per instr, 0.5 cyc/row)
USE_ARS = True       # Abs_reciprocal_sqrt fused rsqrt
USE_BCAST = True     # stride-0 broadcast in the final scale
USE_SWDGE = True     # Pool-queue (GPSIMD software DGE) DMAs
USE_TBF16 = True     # bf16 PSUM transposes (else f32)


def _choose_t_eff(A64):
    for t_eff in (64, 128, 256, 512):
        nrm = np.linalg.norm(np.linalg.matrix_power(A64, t_eff), 2)
        if nrm * T < 1e-9:
            return t_eff
    return 512


def _build_bass(t_eff, consts):
    L2 = t_eff // L1
    TOK = B_LOC * t_eff
    NH = TOK // 128           # token halves
    NZ = D // FCH             # z feature chunks
    bias_act = float(SP * SP * (LN_EPS + consts["bb"] / D))
    scale_act = float(SP * SP / (SW * SW * D))
    bb, bbar = consts["bb"], consts["bbar"]

    nc = bacc.Bacc("TRN2", target_bir_lowering=False)

    xa_d = nc.dram_tensor("xa", [128, 6, TOK], F8, kind="ExternalInput")
    xb_d = nc.dram_tensor("xb", [128, 6, TOK], F8, kind="ExternalInput")
    w_d = [nc.dram_tensor(f"w{p}", [128, 2, D], F8, kind="ExternalInput")
           for p in range(3)]
    sb_d = nc.dram_tensor("sb", [128, 12, 68], F8, kind="ExternalInput")
    sbs_d = nc.dram_tensor("sbs", [128, 6, 68], F8, kind="ExternalInput")
    c64_d = nc.dram_tensor("c64", [N, (L1 + L2) * N + D + N + 4], BF16,
                           kind="ExternalInput")
    c128_d = nc.dram_tensor("c128", [128, 448], BF16, kind="ExternalInput")
    idf_d = nc.dram_tensor("idf", [128, 128], F32, kind="ExternalInput")
    out_d = nc.dram_tensor("out", [128, 6, B_LOC], F32, kind="ExternalOutput")

    DR = mybir.MatmulPerfMode.DoubleRow
    AL = mybir.AluOpType

    with tile.TileContext(nc) as tc:
        with (
            tc.tile_pool(name="const", bufs=1) as const,
            tc.tile_pool(name="small", bufs=24) as small,
            tc.tile_pool(name="ps", bufs=8, space="PSUM") as ps,
        ):
            xt = const.tile([128, 12, TOK], F8, tag="xt")
            wt = const.tile([128, 6, D], F8, tag="wt")
            st = const.tile([128, 12, 68], F8, tag="st")
            sts = const.tile([128, 6, 68], F8, tag="sts")
            c64 = const.tile([N, (L1 + L2) * N + D + N + 4], BF16, tag="c64")
            c128 = const.tile([128, 448], BF16, tag="c128")
            idf = const.tile([128, 128], F32, tag="idf")
            zsq = const.tile([128, 3 * NH * NZ, FCH], BF16, tag="zsq")
            b_eps = const.tile([128, 1], F32, tag="b_eps")
            nc.gpsimd.memset(b_eps, bias_act)
            b_tiny = const.tile([1, 1], F32, tag="b_tiny")
            nc.gpsimd.memset(b_tiny, 1e-24)

            # ---- DMAs.  SP queue -> shared HWDGE (serialized); Pool queue
            # -> SWDGE (parallel to HWDGE).  Wire is shared either way.
            # Pool issues are delayed by a filler memset so the const blobs
            # hit the wire after the critical W chunks. ----
            nc.sync.dma_start(out=xt[:, 0:6, :], in_=xa_d[:, :, :])
            nc.sync.dma_start(out=wt[:, 0:2, :], in_=w_d[0][:, :, :])
            nc.sync.dma_start(out=wt[:, 2:4, :], in_=w_d[1][:, :, :])
            nc.sync.dma_start(out=wt[:, 4:6, :], in_=w_d[2][:, :, :])
            nc.sync.dma_start(out=c128, in_=c128_d[:, :])
            nc.sync.dma_start(out=xt[:, 6:12, :], in_=xb_d[:, :, :])
            nc.sync.dma_start(out=c64, in_=c64_d[:, :])
            if not USE_TBF16:
                nc.scalar.dma_start(out=idf, in_=idf_d[:, :])
            if USE_SWDGE:
                for slab in range(POOL_SLABS):
                    nc.gpsimd.memset(zsq[:, slab, :], 0.0)
                nc.gpsimd.dma_start(out=sts, in_=sbs_d[:, :, :])
                nc.gpsimd.dma_start(out=st, in_=sb_d[:, :, :])
            else:
                nc.scalar.dma_start(out=sts, in_=sbs_d[:, :, :])
                nc.scalar.dma_start(out=st, in_=sb_d[:, :, :])

            # PE pstate warm-up: pe_busy_start is pinned by the first PE
            # activity, and the engine reaches full clock 3us later.  A
            # single dummy matmul on a memset tile at ~0.6us makes every
            # real matmul (starting ~4us) run at full speed.
            dmy = const.tile([128, 8], BF16, tag="dmy")
            nc.vector.memset(dmy, 0.0)
            dmy_ps = ps.tile([8, 8], F32, tag="ps", name="dmy_ps")
            nc.tensor.matmul(out=dmy_ps, lhsT=dmy, rhs=dmy,
                             start=True, stop=True)

            # Dummy Sqrt: forces the single act-table load (set containing
            # both Sqrt and Square) to happen here, off the critical path.
            dmy_a = const.tile([1, 1], F32, tag="dmy_a")
            nc.scalar.activation(
                out=dmy_a, in_=b_tiny,
                func=(mybir.ActivationFunctionType.Abs_reciprocal_sqrt
                      if USE_ARS else mybir.ActivationFunctionType.Sqrt),
                bias=b_tiny[:, :], scale=1.0)

            # const views
            apow1 = c64[:, 0:L1 * N]
            apow2 = c64[:, L1 * N:(L1 + L2) * N]
            cmat = c64[:, (L1 + L2) * N:(L1 + L2) * N + D]
            ccb = c64[:, (L1 + L2) * N + D:(L1 + L2) * N + D + N]
            ones_col = c64[:, (L1 + L2) * N + D + N + 1:(L1 + L2) * N + D + N + 2]
            ident = c128[:, 0:128]
            ones_row = c128[0:1, 128:256]
            c1_row = c128[0:1, 256:320]
            gv_row = c128[0:1, 320:384]
            hconst_row = c128[0:1, 384:448]

            # x views: slots 0-5 = x8 d-tiles, 6-11 = xr d-tiles
            x8v = xt[:, 0:6, :].rearrange("p (q two) t -> p q two t", q=3, two=2)
            xmix = xt[:, :, :].rearrange("p (g d) t -> p g d t", g=2, d=6)
            shi = st[:, 0:12, :].rearrange("p (g d) c -> p g d c", g=2, d=6)
            slo = sts[:, :, :].rearrange("p (q two) c -> p q two c", q=3, two=2)

            # ---- stage 1: z = x8 @ W8^T (fp8 DoubleRow), token-major.
            # Stats matmuls are issued between pair 1 and pair 2 so PE fills
            # the gap while the last W chunk is still in flight. ----
            z_ps = [[ps.tile([128, FCH], F32, tag="ps", name=f"z{h}{f}")
                     for f in range(NZ)] for h in range(NH)]

            def dr_matmul(out, lhsT, rhs, start, stop):
                if USE_DR:
                    nc.tensor.matmul(out=out, lhsT=lhsT, rhs=rhs,
                                     start=start, stop=stop, perf_mode=DR,
                                     skip_group_check=True)
                else:
                    for i in range(2):
                        nc.tensor.matmul(
                            out=out, lhsT=lhsT[:, i, :], rhs=rhs[:, i, :],
                            start=(start and i == 0), stop=(stop and i == 1),
                            skip_group_check=True)

            def z_pair(p):
                for h in range(NH):
                    hsl = slice(h * 128, (h + 1) * 128)
                    for f in range(NZ):
                        dr_matmul(
                            z_ps[h][f], x8v[:, p, :, hsl],
                            wt[:, 2 * p:2 * p + 2, f * FCH:(f + 1) * FCH],
                            p == 0, p == 2)

            # stats from x8 only (mu/xwb need no residual): first thing
            # PE does once x8+sb land, before W arrives.
            qs_ps = [ps.tile([128, 2], F32, tag="ps", name=f"qs{h}")
                     for h in range(NH)]
            qv_ps = [ps.tile([128, 64], F32, tag="ps", name=f"qv{h}")
                     for h in range(NH)]
            z_pair(0)
            z_pair(1)
            z_pair(2)

            for h in range(NH):
                hsl = slice(h * 128, (h + 1) * 128)
                for q in range(3):
                    dr_matmul(qs_ps[h], x8v[:, q, :, hsl],
                              sts[:, 2 * q:2 * q + 2, 64:66],
                              q == 0, q == 2)
            for h in range(NH):
                hsl = slice(h * 128, (h + 1) * 128)
                for q in range(3):
                    dr_matmul(qv_ps[h], x8v[:, q, :, hsl],
                              slo[:, q, :, 0:64], q == 0, False)

            # v hi group: mixed k-pairs (x8_d, xr_d) x (S_hi_d, S_hi16_d),
            # waits for the xr DMA (late, off the critical path)
            for h in range(NH):
                hsl = slice(h * 128, (h + 1) * 128)
                for d in range(6):
                    dr_matmul(qv_ps[h], xmix[:, :, d, hsl],
                              shi[:, :, d, 0:64], False, False)

            # ---- ssq = row-sum(z^2): ACT takes f=0 tiles, DVE f=1 ----
            pp = small.tile([128, NH * NZ], F32, tag="pp")
            ppv = pp[:, :].rearrange("p (h f) -> p h f", h=NH, f=NZ)

            # mu chain on Pool: DVE is reserved for the z square-reduces.
            # msq' = (q*k1 + k2)^2 = D*SW^2*mu^2; preD = xwb*SW^2/SWB - msq'
            k1 = float(np.sqrt(D) * SW / SM)
            k2 = float(np.sqrt(D) * SW * bbar)
            mu_t = small.tile([128, NH], F32, tag="mu_t")
            for h in range(NH):
                nc.vector.tensor_scalar(
                    out=mu_t[:, h:h + 1], in0=qs_ps[h][:, 0:1],
                    scalar1=k1, scalar2=k2, op0=AL.mult, op1=AL.add)
            msq = small.tile([128, NH], F32, tag="msq")
            nc.vector.tensor_tensor(out=msq, in0=mu_t, in1=mu_t, op=AL.mult)
            preD = small.tile([128, NH], F32, tag="preD")
            for h in range(NH):
                nc.vector.scalar_tensor_tensor(
                    out=preD[:, h:h + 1], in0=qs_ps[h][:, 1:2],
                    scalar=SW * SW / SWB, in1=msq[:, h:h + 1],
                    op0=AL.mult, op1=AL.subtract)

            # z tiles complete in order t0..t3 (80ns apart).  ACT
            # square+accums t0 and t2 straight from PSUM; DVE (which cannot
            # square a PSUM operand) copies t1/t3 to SBUF bf16, then DVE
            # square-reduces t3 while Pool square-reduces t1.
            tiles = [z_ps[0][0], z_ps[0][1], z_ps[1][0], z_ps[1][1]]
            idxs = [0 * NZ + 0, 0 * NZ + 1, 1 * NZ + 0, 1 * NZ + 1]
            for i in (0, 1):
                nc.vector.tensor_copy(out=zsq[:, 4 + idxs[i], :],
                                      in_=tiles[i])
            for i in (2, 3):
                nc.scalar.activation(
                    out=zsq[:, idxs[i], :], in_=tiles[i],
                    func=mybir.ActivationFunctionType.Square,
                    accum_out=pp[:, idxs[i]:idxs[i] + 1])
            for i, eng in ((0, nc.vector), (1, nc.vector)):
                eng.scalar_tensor_tensor(
                    out=zsq[:, idxs[i], :], in0=zsq[:, 4 + idxs[i], :],
                    scalar=1.0, in1=zsq[:, 4 + idxs[i], :],
                    op0=AL.mult, op1=AL.mult,
                    accum_out=pp[:, idxs[i]:idxs[i] + 1])

            # c1 fold (the -mu*gv term is folded into P1/c1 on the host)
            for h in range(NH):
                nc.tensor.matmul(
                    out=qv_ps[h], lhsT=ones_row, rhs=c1_row,
                    start=False, stop=True, skip_group_check=True)

            # var -> std -> s  (all [128, NH])
            add1 = small.tile([128, NH], F32, tag="add1")
            nc.vector.tensor_tensor(
                out=add1, in0=ppv[:, :, 0], in1=ppv[:, :, 1], op=AL.add)
            vargD = small.tile([128, NH], F32, tag="vargD")
            nc.vector.tensor_tensor(out=vargD, in0=add1, in1=preD, op=AL.add)
            s_col = small.tile([128, NH], F32, tag="s_col")
            if USE_ARS:
                nc.scalar.activation(
                    out=s_col, in_=vargD,
                    func=mybir.ActivationFunctionType.Abs_reciprocal_sqrt,
                    bias=b_eps[:, :], scale=scale_act)
            else:
                stdS = small.tile([128, NH], F32, tag="stdS")
                nc.scalar.activation(
                    out=stdS, in_=vargD,
                    func=mybir.ActivationFunctionType.Sqrt,
                    bias=b_eps[:, :], scale=scale_act)
                nc.vector.reciprocal(out=s_col, in_=stdS)

            # ---- u = s * v', transpose to feature-major ----
            u_sb = small.tile([128, NH, 64],
                              BF16 if USE_TBF16 else F32, tag="u_sb")
            nc.scalar.activation(
                out=u_sb[:, 0, :], in_=qv_ps[0],
                func=mybir.ActivationFunctionType.Copy,
                bias=0.0, scale=s_col[:, 0:1])
            nc.vector.tensor_scalar_mul(
                out=u_sb[:, 1, :], in0=qv_ps[1], scalar1=s_col[:, 1:2])
            wT_sb = small.tile([N, TOK], BF16, tag="wT_sb")
            uT_ps = ps.tile([N, NH, 128], BF16 if USE_TBF16 else F32,
                            tag="ps", name="uT")
            for h in range(NH):
                nc.tensor.transpose(uT_ps[:, h, :], u_sb[:, h, :],
                                    ident if USE_TBF16 else idf)
            nc.vector.tensor_copy(out=wT_sb, in_=uT_ps)

            # ---- chunked scan ----
            wT_v = wT_sb[:, :].rearrange("n (b j l) -> n b j l",
                                         b=B_LOC, j=L2, l=L1)
            s1_ps = ps.tile([N, B_LOC, L2], F32, tag="ps")
            for l in range(L1):
                nc.tensor.matmul(
                    out=s1_ps, lhsT=apow1[:, l * N:(l + 1) * N],
                    rhs=wT_v[:, :, :, l], start=(l == 0), stop=(l == L1 - 1))
            s_sb = small.tile([N, B_LOC, L2], BF16, tag="s_sb")
            nc.vector.tensor_copy(out=s_sb, in_=s1_ps)
            h_ps = ps.tile([N, B_LOC], F32, tag="ps")
            for j in range(L2):
                nc.tensor.matmul(
                    out=h_ps, lhsT=apow2[:, j * N:(j + 1) * N],
                    rhs=s_sb[:, :, j], start=(j == 0), stop=False,
                    skip_group_check=True)
            nc.tensor.matmul(
                out=h_ps, lhsT=hconst_row, rhs=ones_row[:, 0:B_LOC],
                start=False, stop=True, skip_group_check=True)
            h_sb = small.tile([N, B_LOC], BF16, tag="h_sb")
            nc.vector.tensor_copy(out=h_sb, in_=h_ps)

            # ---- y^T = C^T h (unscaled) overlaps the norm chain ----
            y_ps = ps.tile([128, 6, B_LOC], F32, tag="ps")
            for c in range(6):
                nc.tensor.matmul(
                    out=y_ps[:, c, :], lhsT=cmat[:, c * 128:(c + 1) * 128],
                    rhs=h_sb, start=True, stop=True, skip_group_check=True)

            # ---- norm: 1/||y|| via CC, folded in at the output copy ----
            hcc_ps = ps.tile([N, B_LOC], F32, tag="ps")
            nc.tensor.matmul(out=hcc_ps, lhsT=ccb, rhs=h_sb,
                             start=True, stop=True)
            prod2 = small.tile([N, B_LOC], BF16, tag="prod2")
            nc.vector.tensor_tensor(out=prod2, in0=h_sb, in1=hcc_ps,
                                    op=AL.mult)
            ssum_ps = ps.tile([1, B_LOC], F32, tag="ps")
            nc.tensor.matmul(out=ssum_ps, lhsT=ones_col, rhs=prod2,
                             start=True, stop=True)
            rn_row = small.tile([1, B_LOC], BF16, tag="rn_row")
            if USE_ARS:
                nc.scalar.activation(
                    out=rn_row, in_=ssum_ps,
                    func=mybir.ActivationFunctionType.Abs_reciprocal_sqrt,
                    bias=b_tiny[:, :], scale=1.0)
            else:
                nrm_row = small.tile([1, B_LOC], F32, tag="nrm_row")
                nc.scalar.activation(
                    out=nrm_row, in_=ssum_ps,
                    func=mybir.ActivationFunctionType.Sqrt,
                    bias=b_tiny[:, :], scale=1.0)
                with nc.allow_low_precision(reason="bf16 1/nrm"):
                    nc.vector.reciprocal(out=rn_row, in_=nrm_row)
            y_sb = small.tile([128, 6, B_LOC], F32, tag="y_sb")
            if USE_BCAST:
                rn128_ps = ps.tile([128, B_LOC], F32, tag="ps")
                nc.tensor.matmul(out=rn128_ps, lhsT=ones_row, rhs=rn_row,
                                 start=True, stop=True)
                rn128_sb = small.tile([128, B_LOC], F32, tag="rn128_sb")
                nc.vector.tensor_copy(out=rn128_sb, in_=rn128_ps)
                nc.vector.tensor_tensor(
                    out=y_sb, in0=y_ps,
                    in1=rn128_sb[:, :].unsqueeze(1).to_broadcast(
                        (128, 6, B_LOC)),
                    op=AL.mult)
            else:
                rn64_ps = ps.tile([N, B_LOC], F32, tag="ps")
                nc.tensor.matmul(out=rn64_ps, lhsT=ones_row[:, 0:64],
                                 rhs=rn_row, start=True, stop=True)
                h2 = small.tile([N, B_LOC], BF16, tag="h2")
                nc.vector.tensor_tensor(out=h2, in0=h_sb, in1=rn64_ps,
                                        op=AL.mult)
                y2_ps = ps.tile([128, 6, B_LOC], F32, tag="ps")
                for c in range(6):
                    nc.tensor.matmul(
                        out=y2_ps[:, c, :],
                        lhsT=cmat[:, c * 128:(c + 1) * 128],
                        rhs=h2, start=True, stop=True,
                        skip_group_check=True)
                nc.vector.tensor_copy(out=y_sb, in_=y2_ps)
            nc.sync.dma_start(out=out_d[:, :, :], in_=y_sb)

    if not nc.is_finalized():
        nc.finalize()
    return nc


def prepare(inputs):
    """Host-side derived weights (fp64) and fp8/bf16 packing."""
    import ml_dtypes
    F8N = ml_dtypes.float8_e4m3
    BFN = ml_dtypes.bfloat16
    f64 = np.float64

    W = np.asarray(inputs["W_lin"], f64)
    b = np.asarray(inputs["b_lin"], f64)
    g = np.asarray(inputs["gamma"], f64)
    be = np.asarray(inputs["beta"], f64)
    A = np.asarray(inputs["A"], f64)
    Bm = np.asarray(inputs["Bm"], f64)
    C = np.asarray(inputs["C"], f64)

    t_eff = _choose_t_eff(A)
    L2 = t_eff // L1

    G = g[:, None] * Bm
    gv_ = g @ Bm
    m_ = W.sum(axis=0) / D
    P1 = W.T @ G - np.outer(m_, gv_)
    c1 = b @ G
    m = W.sum(axis=0) / D
    bbar = float(b.mean())
    wb = W.T @ b
    bb = float(b @ b)
    gv = g @ Bm
    bbeta = be @ Bm

    Asum = np.zeros((N, N))
    Ak = np.eye(N)
    for _ in range(t_eff):
        Asum += Ak
        Ak = Ak @ A
    hconst = bbeta @ Asum

    def q8(a):
        return np.asarray(a, np.float32).astype(F8N)

    # W^T in fp8, k-pair layout
    WT8 = q8((W * SW).T)                       # [768 d, 768 f]
    w_pairs = [np.ascontiguousarray(
        WT8[256 * p:256 * (p + 1), :].reshape(2, 128, D).transpose(1, 0, 2))
        for p in range(3)]

    # stats weights with residual
    S_hi = np.concatenate(
        [P1 * SP, (m * SM)[:, None], (2.0 * wb * SWB)[:, None]], axis=1)
    S8 = q8(S_hi)
    S_hi16 = q8(S_hi / SR)
    S_lo16 = q8(np.asarray(q8((S_hi - np.asarray(S8, f64)) * SRP),
                           f64) / SRP)
    sbs = np.zeros((128, 6, 68), F8N)
    sblob = np.zeros((128, 12, 68), F8N)
    for d in range(6):
        rows = slice(d * 128, (d + 1) * 128)
        sblob[:, d, 0:66] = S8[rows, :]
        sblob[:, 6 + d, 0:66] = S_hi16[rows, :]
        sbs[:, d, 0:64] = S_lo16[rows, 0:64]
        sbs[:, d, 64:66] = S8[rows, 64:66]

    # bf16 const blobs
    Apows = [np.eye(N)]
    for _ in range(L1):
        Apows.append(Apows[-1] @ A)
    apow1 = np.concatenate([Apows[L1 - 1 - l] for l in range(L1)], axis=1)
    A_L1 = Apows[L1]
    apow2 = np.concatenate(
        [np.linalg.matrix_power(A_L1, L2 - 1 - j) for j in range(L2)], axis=1)
    CC = C @ C.T
    c64 = np.zeros((N, (L1 + L2) * N + D + N + 4), BFN)
    o = 0
    c64[:, o:o + L1 * N] = apow1.astype(BFN); o += L1 * N
    c64[:, o:o + L2 * N] = apow2.astype(BFN); o += L2 * N
    c64[:, o:o + D] = C.astype(BFN); o += D
    c64[:, o:o + N] = CC.astype(BFN); o += N
    c64[:, o] = hconst.astype(BFN); o += 1
    c64[:, o] = 1.0

    c128 = np.zeros((128, 448), BFN)
    c128[0:128, 0:128] = np.eye(128, dtype=BFN)
    c128[0, 128:256] = 1.0
    c128[0, 256:320] = (SP * (c1 - bbar * gv)).astype(BFN)
    c128[0, 384:448] = hconst.astype(BFN)

    return {
        "t_eff": t_eff,
        "w_pairs": w_pairs,
        "sblob": np.ascontiguousarray(sblob),
        "sbs": np.ascontiguousarray(sbs),
        "c64": np.ascontiguousarray(c64),
        "c128": np.ascontiguousarray(c128),
        "consts": {"bb": bb, "bbar": bbar},
    }


def make_in_maps(x, prep):
    import ml_dtypes
    F8N = ml_dtypes.float8_e4m3
    t_eff = prep["t_eff"]
    TOK = B_LOC * t_eff
    in_maps = []
    for core in range(N_CORES):
        xs = np.asarray(
            x[core * B_LOC:(core + 1) * B_LOC, T - t_eff:, :], np.float32)
        xT = np.ascontiguousarray(xs.reshape(TOK, D).T)      # [768, TOK]
        x8 = xT.astype(F8N)
        xr = ((xT - np.asarray(x8, np.float32)) * SR).astype(F8N)
        x8t = x8.reshape(6, 128, TOK).transpose(1, 0, 2)     # [128, 6, TOK]
        xrt = xr.reshape(6, 128, TOK).transpose(1, 0, 2)
        m = {
            "xa": np.ascontiguousarray(x8t),
            "xb": np.ascontiguousarray(xrt),
            "sb": prep["sblob"],
            "sbs": prep["sbs"],
            "c64": prep["c64"],
            "c128": prep["c128"],
            "idf": np.eye(128, dtype=np.float32),
        }
        for p in range(3):
            m[f"w{p}"] = prep["w_pairs"][p]
        in_maps.append(m)
    return in_maps


def kernel(x, W_lin, b_lin, gamma, beta, A, Bm, C):
    global LAST_RESULTS, LAST_NC
    x = np.asarray(x, np.float32)
    assert x.shape == (B, T, D), x.shape

    prep = prepare(dict(W_lin=W_lin, b_lin=b_lin, gamma=gamma, beta=beta,
                        A=A, Bm=Bm, C=C))
    nc = _build_bass(prep["t_eff"], prep["consts"])
    in_maps = make_in_maps(x, prep)

    LAST_NC = nc
    res = run_bass_kernel_spmd(nc, in_maps, core_ids=list(range(N_CORES)))
    LAST_RESULTS = res
    outs = []
    for r in res.results:
        yT = r["out"]                       # [128, 6, B_LOC]
        outs.append(yT.transpose(2, 1, 0).reshape(B_LOC, D))
    return np.concatenate(outs, axis=0).astype(np.float32)
